# revision 11
# baseline (speedup 1.0000x reference)
"""NNConv+GRU message-passing network (ConvGRU) on 8 Trainium2 NeuronCores.

v2: mixed-path per-edge contraction, balancing PE / ACT / DVE / GPSIMD:

  msg[e,o] = sum_i h[src[e],i] * We[e,i,o],  We = edge-MLP(edge_attr)

  - BASE path (oi-chunks q < Q_BASE, edge-major): We tile [e128, 1024] = PE
    (hidT-tile stationary), ACT evacuates PSUM->bf16, DVE (or GPSIMD)
    broadcast-multiplies by gathered h, DVE tensor_reduce sums i.
  - V1 path (chunks c >= 8*Q_BASE, (o,i)-major): WeT chunk [(o2,i64), e] = PE
    (w2p chunk stationary), PSUM evacuated by ACT (or read directly by DVE),
    multiplied by the xbar-transposed+duplicated gathered h, then reduced
    over i by a PE mask-matmul into msgT rows; msgT is xbar-transposed back
    into the edge-major msg tiles.  KS chunks are staged (MM+evac) into SBUF
    during the previous AllGather window.

  Scatter-add to nodes = per-tile selection matmul (S built on-chip from dst
  indices).  Everything bf16 except GRU state math and pooling accumulation.
"""
import numpy as np

DIM = 64
DEPTHS = 3
N_NODES = 8192
N_EDGES = 16384
N_GRAPHS = 64
NC = 8
NPC = N_NODES // NC
P = 128

Q_BASE = 2        # oi-1024-chunks on the BASE path (edge-major, DVE reduce)
KS = 5            # staged V1 chunks (MM+evac pre-run during AllGather window)
RMM_LAG = 2       # software pipelining for the V1 mask-matmul
GPS_EVERY = 3     # every GPS_EVERY-th base (t,q) multiply goes to GPSIMD
DIR_EVERY = 3     # every DIR_EVERY-th in-loop V1 chunk multiplies from PSUM

TRACE = False
LAST_EXEC_NS = None
LAST_RESULTS = None

_CACHE = {}


def _build(T, b2_zero):
    import concourse.mybir as mybir
    import concourse.tile as tile
    from concourse import bacc
    import concourse.bass as bass
    from concourse.masks import make_identity

    f32 = mybir.dt.float32
    bf16 = mybir.dt.bfloat16
    i32 = mybir.dt.int32
    AF = mybir.ActivationFunctionType
    OP = mybir.AluOpType
    EP = T * P

    q_base = Q_BASE if b2_zero else 0  # base path has no bias support
    v1_c0 = 8 * q_base
    ks = min(KS, 32 - v1_c0)

    echunks = []
    t0 = 0
    while t0 < T:
        nt = min(8, T - t0)
        echunks.append((t0, nt))
        t0 += nt

    nc = bacc.Bacc("TRN2", target_bir_lowering=False, debug=False, num_devices=NC)

    def din(name, shape, dt=f32):
        return nc.dram_tensor(name, shape, dt, kind="ExternalInput")

    xT_d = din("xT", [40, NPC])
    eaT_d = din("eaT", [10, EP], bf16)
    srcx_d = din("srcidx", [P, T], i32)
    dsti_d = din("dsti", [P, T])
    rmask_d = din("rmask", [P, 32 * 64], bf16)
    pS_d = din("poolS", [NPC, N_GRAPHS], bf16)
    fc0_wT_d = din("fc0_wT", [40, 32])
    fc0_b_d = din("fc0_b", [32, 1])
    g0_wihT_d = din("g0_wihT", [32, 192])
    g0_brz_d = din("g0_brz", [128, 1])
    g0_bihn_d = din("g0_bihn", [64, 1])
    g0_bhhn_d = din("g0_bhhn", [64, 1])
    w2p_d = [din(f"w2p{d}", [128, 4096], bf16) for d in range(DEPTHS)]
    m1wT_d = [din(f"m1wT{d}", [10, 128], bf16) for d in range(DEPTHS)]
    m1b_d = [din(f"m1b{d}", [128, 1]) for d in range(DEPTHS)]
    root_d = [din(f"root{d}", [64, 64], bf16) for d in range(DEPTHS)]
    convb_d = [din(f"convb{d}", [64, 1]) for d in range(DEPTHS)]
    wihT_d = [din(f"wihT{d}", [64, 192], bf16) for d in range(DEPTHS)]
    whhT_d = [din(f"whhT{d}", [64, 192], bf16) for d in range(DEPTHS)]
    brz_d = [din(f"brz{d}", [128, 1]) for d in range(DEPTHS)]
    bihn_d = [din(f"bihn{d}", [64, 1]) for d in range(DEPTHS)]
    bhhn_d = [din(f"bhhn{d}", [64, 1]) for d in range(DEPTHS)]
    b2t_d = None if b2_zero else [din(f"b2t{d}", [128, 32]) for d in range(DEPTHS)]
    o0wT_d = din("o0wT", [64, 64])
    o0b_d = din("o0b", [64, 1])
    o1wT_d = din("o1wT", [64, 32])
    o1b_d = din("o1b", [32, 1])
    o2wT_d = din("o2wT", [32, 1])
    o2b_d = din("o2b", [1, 1])

    y_d = nc.dram_tensor("y", [1, N_GRAPHS], f32, kind="ExternalOutput")

    RG = [list(range(NC))]

    with tile.TileContext(nc) as tc:
        with (
            tc.tile_pool(name="const", bufs=1) as cp,
            tc.tile_pool(name="work", bufs=1) as wp,
            tc.tile_pool(name="edge", bufs=6) as ep,
            tc.tile_pool(name="prod", bufs=3) as pp,
            tc.tile_pool(name="msgp", bufs=12) as mp,
            tc.tile_pool(name="weps", bufs=2, space="PSUM") as pwe,
            tc.tile_pool(name="pmsg", bufs=1, space="PSUM") as pmsg,
            tc.tile_pool(name="pagg", bufs=1, space="PSUM") as pagg,
            tc.tile_pool(name="dram", bufs=1, space="DRAM") as dp,
        ):
            def load(name, dram, shape, dt=f32, eng=None):
                t = cp.tile(shape, dt, name=name)
                (eng or nc.gpsimd).dma_start(t[:], dram[:, :])
                return t

            # phase0-critical loads on the scalar queue, bulk on gpsimd
            xT = load("xT_s", xT_d, [40, NPC], eng=nc.scalar)
            fc0_wT = load("fc0_wT_s", fc0_wT_d, [40, 32], eng=nc.scalar)
            fc0_b = load("fc0_b_s", fc0_b_d, [32, 1], eng=nc.scalar)
            g0_wihT = load("g0_wihT_s", g0_wihT_d, [32, 192], eng=nc.scalar)
            g0_brz = load("g0_brz_s", g0_brz_d, [128, 1], eng=nc.scalar)
            g0_bihn = load("g0_bihn_s", g0_bihn_d, [64, 1], eng=nc.scalar)
            g0_bhhn = load("g0_bhhn_s", g0_bhhn_d, [64, 1], eng=nc.scalar)
            srcx = load("srcx_s", srcx_d, [P, T], i32, eng=nc.scalar)
            dsti = load("dsti_s", dsti_d, [P, T], eng=nc.scalar)
            eaT = load("eaT_s", eaT_d, [10, EP], bf16, eng=nc.scalar)
            m1wT = [load(f"m1wT_s{d}", m1wT_d[d], [10, 128], bf16, eng=nc.scalar)
                    for d in range(DEPTHS)]
            m1b = [load(f"m1b_s{d}", m1b_d[d], [128, 1], eng=nc.scalar)
                   for d in range(DEPTHS)]
            rmask = load("rmask_s", rmask_d, [P, 32 * 64], bf16)
            w2p = [load(f"w2p_s{d}", w2p_d[d], [128, 4096], bf16) for d in range(DEPTHS)]
            rootw = [load(f"root_s{d}", root_d[d], [64, 64], bf16) for d in range(DEPTHS)]
            convb = [load(f"convb_s{d}", convb_d[d], [64, 1]) for d in range(DEPTHS)]
            wihT = [load(f"wihT_s{d}", wihT_d[d], [64, 192], bf16) for d in range(DEPTHS)]
            whhT = [load(f"whhT_s{d}", whhT_d[d], [64, 192], bf16) for d in range(DEPTHS)]
            brz = [load(f"brz_s{d}", brz_d[d], [128, 1]) for d in range(DEPTHS)]
            bihn = [load(f"bihn_s{d}", bihn_d[d], [64, 1]) for d in range(DEPTHS)]
            bhhn = [load(f"bhhn_s{d}", bhhn_d[d], [64, 1]) for d in range(DEPTHS)]
            b2t = (
                None if b2_zero else
                [load(f"b2t_s{d}", b2t_d[d], [128, 32]) for d in range(DEPTHS)]
            )
            o0wT = load("o0wT_s", o0wT_d, [64, 64])
            o0b = load("o0b_s", o0b_d, [64, 1])
            o1wT = load("o1wT_s", o1wT_d, [64, 32])
            o1b = load("o1b_s", o1b_d, [32, 1])
            o2wT = load("o2wT_s", o2wT_d, [32, 1])
            o2b = load("o2b_s", o2b_d, [1, 1])
            pS = cp.tile([P, 8 * N_GRAPHS], bf16, name="pS_s")
            for c in range(8):
                nc.gpsimd.dma_start(
                    pS[:, c * N_GRAPHS:(c + 1) * N_GRAPHS],
                    pS_d[c * P:(c + 1) * P, :],
                )

            # S selection matrix on-chip: S[e, t*NPC+n] = (dsti[e,t]==n)
            iot = cp.tile([P, NPC], f32, name="iot")
            nc.gpsimd.iota(iot[:], pattern=[[1, NPC]], base=0, channel_multiplier=0,
                           allow_small_or_imprecise_dtypes=True)
            S = cp.tile([P, T * NPC], bf16, name="S_s")
            for t in range(T):
                nc.vector.tensor_scalar(
                    out=S[:, t * NPC:(t + 1) * NPC], in0=iot[:],
                    scalar1=dsti[:, t:t + 1], scalar2=None, op0=OP.is_equal,
                )

            hown = [dp.tile([NPC, DIM], bf16, name=f"hown{d}") for d in range(DEPTHS)]
            hfull = [dp.tile([N_NODES, DIM], bf16, name=f"hfull{d}") for d in range(DEPTHS)]
            ar_in = dp.tile([DIM, N_GRAPHS], f32, name="ar_in")
            ar_out = dp.tile([DIM, N_GRAPHS], f32, name="ar_out")

            hidT = [cp.tile([P, EP], bf16, name=f"hidT{d}") for d in range(DEPTHS)]
            westg = [cp.tile([P, EP], bf16, name=f"westg{c}") for c in range(ks)]

            _nctr = [0]

            def psum_mm(lhsT, rhs_fn, m, n_total, out_fn, nmax=512):
                off = 0
                while off < n_total:
                    n = min(nmax, n_total - off)
                    _nctr[0] += 1
                    w = pwe.tile([P, 1024], f32, name=f"w{_nctr[0]}", tag="mm")
                    if n > 512:
                        nc.tensor.matmul(w[0:m, 0:512], lhsT, rhs_fn(off, 512),
                                         start=True, stop=True)
                        nc.tensor.matmul(w[0:m, 512:n], lhsT, rhs_fn(off + 512, n - 512),
                                         start=True, stop=True)
                    else:
                        nc.tensor.matmul(w[0:m, 0:n], lhsT, rhs_fn(off, n),
                                         start=True, stop=True)
                    out_fn(w, off, n)
                    off += n

            def gru_elem(rz_s, gi_n_s, hn_s, h_prev, tagp):
                z_s = wp.tile([64, NPC], f32, name=f"z_{tagp}", tag="gru_z")
                nc.sync.dma_start(z_s[:], rz_s[64:128, :])
                t1 = wp.tile([64, NPC], f32, name=f"t1_{tagp}", tag="gru_t1")
                nc.vector.tensor_tensor(out=t1[:], in0=rz_s[0:64, :], in1=hn_s[:], op=OP.mult)
                nc.vector.tensor_tensor(out=t1[:], in0=t1[:], in1=gi_n_s[:], op=OP.add)
                nt = wp.tile([64, NPC], f32, name=f"nt_{tagp}", tag="gru_nt")
                nc.scalar.activation(nt[:], t1[:], AF.Tanh)
                hm = wp.tile([64, NPC], f32, name=f"hm_{tagp}", tag="gru_hm")
                if h_prev is None:
                    nc.vector.tensor_tensor(out=hm[:], in0=z_s[:], in1=nt[:], op=OP.mult)
                    hnew = wp.tile([64, NPC], f32, name=f"h_{tagp}", tag="hT")
                    nc.vector.tensor_tensor(out=hnew[:], in0=nt[:], in1=hm[:], op=OP.subtract)
                else:
                    nc.vector.tensor_tensor(out=hm[:], in0=h_prev[:], in1=nt[:], op=OP.subtract)
                    nc.vector.tensor_tensor(out=hm[:], in0=hm[:], in1=z_s[:], op=OP.mult)
                    hnew = wp.tile([64, NPC], f32, name=f"h_{tagp}", tag="hT")
                    nc.vector.tensor_tensor(out=hnew[:], in0=hm[:], in1=nt[:], op=OP.add)
                h16 = wp.tile([64, NPC], bf16, name=f"h16_{tagp}", tag="hT16")
                nc.vector.tensor_copy(h16[:], hnew[:])
                return hnew, h16

            def h_out(h16, d_next, tagp):
                hnm = wp.tile([P, 8 * DIM], bf16, name=f"hnm_{tagp}", tag="hnm")
                for c in range(8):
                    nc.scalar.dma_start_transpose(
                        hnm[:, c * DIM:(c + 1) * DIM], h16[:, c * P:(c + 1) * P]
                    )
                    nc.scalar.dma_start(
                        hown[d_next][c * P:(c + 1) * P, :],
                        hnm[:, c * DIM:(c + 1) * DIM],
                    )
                nc.gpsimd.collective_compute(
                    "AllGather", OP.bypass, replica_groups=RG,
                    ins=[hown[d_next].opt()], outs=[hfull[d_next].opt()],
                )

            def stage_wet(d):
                for j in range(ks):
                    c = v1_c0 + j
                    def evac(wt, off, n, _c=c, _j=j):
                        if b2_zero:
                            nc.scalar.activation(
                                westg[_j][:, off:off + n], wt[:, 0:n], AF.Copy)
                        else:
                            nc.scalar.activation(
                                westg[_j][:, off:off + n], wt[:, 0:n],
                                AF.Identity, bias=b2t[d][:, _c:_c + 1])
                    psum_mm(w2p[d][:, c * P:(c + 1) * P],
                            lambda off, n: hidT[d][:, off:off + n],
                            P, EP, evac, nmax=1024)

            def build_hid(d):
                def evac(wt, off, n, _d=d):
                    nc.scalar.activation(
                        hidT[_d][:, off:off + n], wt[:, 0:n], AF.Relu,
                        bias=m1b[_d][:, 0:1])
                psum_mm(m1wT[d][:], lambda off, n, _d=d: eaT[:, off:off + n],
                        P, EP, evac, nmax=1024)

            # ---------------- phase 0 ----------------
            x0r = wp.tile([32, NPC], f32, name="x0r")
            psum_mm(fc0_wT[:], lambda off, n: xT[:, off:off + n], 32, NPC,
                    lambda wt, off, n: nc.scalar.activation(
                        x0r[:, off:off + n], wt[0:32, 0:n], AF.Relu,
                        bias=fc0_b[:, 0:1]))

            rz0 = wp.tile([P, NPC], f32, name="rz0", tag="gru_rz")
            psum_mm(g0_wihT[:, 0:128], lambda off, n: x0r[:, off:off + n], P, NPC,
                    lambda wt, off, n: nc.scalar.activation(
                        rz0[:, off:off + n], wt[0:128, 0:n], AF.Sigmoid,
                        bias=g0_brz[:, 0:1]))

            gin0 = wp.tile([64, NPC], f32, name="gin0", tag="gru_gin")
            psum_mm(g0_wihT[:, 128:192], lambda off, n: x0r[:, off:off + n], 64, NPC,
                    lambda wt, off, n: nc.scalar.activation(
                        gin0[:, off:off + n], wt[0:64, 0:n], AF.Identity,
                        bias=g0_bihn[:, 0:1]))
            hn0 = wp.tile([64, NPC], f32, name="hn0", tag="gru_hn")
            nc.vector.tensor_scalar_mul(hn0[:], rz0[0:64, :], g0_bhhn[:, 0:1])
            h_T, h16 = gru_elem(rz0, gin0, hn0, None, "p0")
            h_out(h16, 0, "p0")

            for d in range(DEPTHS):
                build_hid(d)
            stage_wet(0)

            # ---------------- conv depths ----------------
            for d in range(DEPTHS):
                # --- phase A: gathers (+ transposed copy for the V1 path) ---
                hsrcT2 = wp.tile([P, EP], bf16, name=f"hsrc{d}", tag="hsrc")
                hsf2s = []
                for pr in range(T // 2):
                    t0, t1 = 2 * pr, 2 * pr + 1
                    hsf2 = ep.tile([P, P], bf16, name=f"hsf{d}_{pr}", tag="hsf", bufs=10)
                    for j, t in enumerate((t0, t1)):
                        nc.gpsimd.indirect_dma_start(
                            out=hsf2[:, j * 64:(j + 1) * 64], out_offset=None,
                            in_=hfull[d][:, :],
                            in_offset=bass.IndirectOffsetOnAxis(
                                ap=srcx[:, t:t + 1], axis=0),
                        )
                    hsf2s.append(hsf2)
                    tmpT = ep.tile([P, P], bf16, name=f"tmpT{d}_{pr}", tag="tmpT")
                    nc.sync.dma_start_transpose(tmpT[:], hsf2[:])
                    for j, t in enumerate((t0, t1)):
                        nc.sync.dma_start(
                            hsrcT2[0:64, t * P:(t + 1) * P], tmpT[j * 64:(j + 1) * 64, :])
                        nc.sync.dma_start(
                            hsrcT2[64:128, t * P:(t + 1) * P], tmpT[j * 64:(j + 1) * 64, :])

                aggT = pagg.tile([64, NPC], f32, name=f"aggT{d}", tag="agg")
                n_sc = [0]

                def scatter_tile(t, msg_t):
                    for s in range(2):
                        nc.tensor.matmul(
                            aggT[0:64, s * 512:(s + 1) * 512],
                            msg_t[:],
                            S[:, t * NPC + s * 512: t * NPC + (s + 1) * 512],
                            start=(n_sc[0] == 0), stop=False,
                        )
                    n_sc[0] += 1

                pend_rmm = []
                pend_scatter = []

                def flush_rmm():
                    msgT_, c_, pt_, cols_ = pend_rmm.pop(0)
                    off = 0
                    while off < cols_:
                        n = min(512, cols_ - off)
                        nc.tensor.matmul(
                            msgT_[0:64, off:off + n],
                            rmask[:, c_ * 64:(c_ + 1) * 64], pt_[:, off:off + n],
                            start=(c_ == v1_c0), stop=(c_ == 31))
                        off += n

                gps_ctr = [0]
                for ci, (tc0, ntc) in enumerate(echunks):
                    cols = ntc * P
                    ec0 = tc0 * P
                    msgtiles = [mp.tile([P, DIM], bf16, name=f"msg{d}_{tc0 + tt}",
                                        tag="msg") for tt in range(ntc)]

                    # ---- BASE path: per (tile, q) edge-major ----
                    for tt in range(ntc):
                        t = tc0 + tt
                        hsb = hsf2s[t // 2][:, (t % 2) * 64:(t % 2 + 1) * 64]
                        for q in range(q_base):
                            _nctr[0] += 1
                            w = pwe.tile([P, 1024], f32, name=f"bw{_nctr[0]}", tag="mm")
                            for s in range(2):
                                nc.tensor.matmul(
                                    w[:, s * 512:(s + 1) * 512],
                                    hidT[d][:, t * P:(t + 1) * P],
                                    w2p[d][:, q * 1024 + s * 512:q * 1024 + (s + 1) * 512],
                                    start=True, stop=True)
                            wsb = pp.tile([P, 1024], bf16, name=f"bwsb{d}_{t}_{q}",
                                          tag="wsb", bufs=3)
                            nc.scalar.activation(wsb[:], w[:], AF.Copy)
                            prod = pp.tile([P, 1024], bf16, name=f"bprod{d}_{t}_{q}",
                                           tag="prod", bufs=3)
                            gps_ctr[0] += 1
                            veng = nc.gpsimd if gps_ctr[0] % GPS_EVERY == 0 else nc.vector
                            veng.tensor_tensor(
                                out=prod[:].rearrange("p (o i) -> p o i", i=64),
                                in0=wsb[:].rearrange("p (o i) -> p o i", i=64),
                                in1=hsb.unsqueeze(1).to_broadcast([P, 16, 64]),
                                op=OP.mult)
                            msgf = wp.tile([P, 16], f32, name=f"msgf{d}_{t}_{q}",
                                           tag="msgf", bufs=4)
                            nc.vector.tensor_reduce(
                                out=msgf[:],
                                in_=prod[:].rearrange("p (o i) -> p o i", i=64),
                                axis=mybir.AxisListType.X, op=OP.add)
                            nc.vector.tensor_copy(
                                msgtiles[tt][:, q * 16:(q + 1) * 16], msgf[:])

                    # ---- V1 path ----
                    msgT = pmsg.tile([64, 1024], f32, name=f"msgT{d}_{ci}", tag="msgT")
                    for c in range(v1_c0, 32):
                        j = c - v1_c0
                        pt = pp.tile([P, 1024], bf16, name=f"pt{d}_{ci}_{c}",
                                     tag="pt", bufs=4)
                        if j < ks:
                            nc.vector.tensor_tensor(
                                out=pt[:, 0:cols],
                                in0=westg[j][:, ec0:ec0 + cols],
                                in1=hsrcT2[:, ec0:ec0 + cols], op=OP.mult)
                        else:
                            direct = ((j - ks) % DIR_EVERY == 1)
                            off = 0
                            while off < cols:
                                n = min(512, cols - off)
                                _nctr[0] += 1
                                w = pwe.tile([P, 1024], f32, name=f"vw{_nctr[0]}",
                                             tag="mm")
                                nc.tensor.matmul(
                                    w[:, 0:n],
                                    w2p[d][:, c * P:(c + 1) * P],
                                    hidT[d][:, ec0 + off:ec0 + off + n],
                                    start=True, stop=True)
                                if direct and b2_zero:
                                    nc.vector.tensor_tensor(
                                        out=pt[:, off:off + n], in0=w[:, 0:n],
                                        in1=hsrcT2[:, ec0 + off:ec0 + off + n],
                                        op=OP.mult)
                                elif direct:
                                    nc.vector.scalar_tensor_tensor(
                                        out=pt[:, off:off + n], in0=w[:, 0:n],
                                        scalar=b2t[d][:, c:c + 1],
                                        in1=hsrcT2[:, ec0 + off:ec0 + off + n],
                                        op0=OP.add, op1=OP.mult)
                                else:
                                    wsb = pp.tile([P, 1024], bf16,
                                                  name=f"vwsb{d}_{ci}_{c}_{off}",
                                                  tag="wsb", bufs=3)
                                    if b2_zero:
                                        nc.scalar.activation(
                                            wsb[:, 0:n], w[:, 0:n], AF.Copy)
                                    else:
                                        nc.scalar.activation(
                                            wsb[:, 0:n], w[:, 0:n], AF.Identity,
                                            bias=b2t[d][:, c:c + 1])
                                    nc.vector.tensor_tensor(
                                        out=pt[:, off:off + n], in0=wsb[:, 0:n],
                                        in1=hsrcT2[:, ec0 + off:ec0 + off + n],
                                        op=OP.mult)
                                off += n
                        pend_rmm.append((msgT, c, pt, cols))
                        if len(pend_rmm) > RMM_LAG:
                            flush_rmm()
                        if c == v1_c0 + 6 and pend_scatter:
                            while pend_scatter:
                                scatter_tile(*pend_scatter.pop(0))
                    while pend_rmm:
                        flush_rmm()

                    # evacuate V1 msgT rows and transpose into msg tiles
                    ob = 2 * v1_c0   # first V1 o-column
                    msgS = wp.tile([64 - ob, 1024], bf16, name=f"msgS{d}_{ci}",
                                   tag="msgS", bufs=2)
                    nc.scalar.activation(msgS[:, 0:cols], msgT[ob:64, 0:cols], AF.Copy)
                    for tt in range(ntc):
                        nc.sync.dma_start_transpose(
                            msgtiles[tt][:, ob:64], msgS[:, tt * P:(tt + 1) * P])
                        pend_scatter.append((tc0 + tt, msgtiles[tt]))
                while pend_scatter:
                    scatter_tile(*pend_scatter.pop(0))

                # root contribution + bias + relu
                for s in range(2):
                    nc.tensor.matmul(
                        aggT[0:64, s * 512:(s + 1) * 512],
                        rootw[d][:],
                        h16[:, s * 512:(s + 1) * 512],
                        start=False, stop=(s == 1),
                    )
                xc = wp.tile([64, NPC], f32, name=f"xc{d}", tag="xc")
                nc.scalar.activation(xc[:], aggT[0:64, :], AF.Relu, bias=convb[d][:, 0:1])
                xc16 = wp.tile([64, NPC], bf16, name=f"xc16{d}", tag="xc16")
                nc.vector.tensor_copy(xc16[:], xc[:])

                # ---- GRU(xc, h) ----
                girz = wp.tile([P, NPC], f32, name=f"girz{d}", tag="gru_girz")
                psum_mm(wihT[d][:, 0:128], lambda off, n: xc16[:, off:off + n], P, NPC,
                        lambda wt, off, n: nc.scalar.activation(
                            girz[:, off:off + n], wt[0:128, 0:n], AF.Copy),
                        nmax=1024)
                rzs = wp.tile([P, NPC], f32, name=f"rzs{d}", tag="gru_rzs")
                psum_mm(whhT[d][:, 0:128], lambda off, n: h16[:, off:off + n], P, NPC,
                        lambda wt, off, n: nc.vector.tensor_tensor(
                            out=rzs[:, off:off + n], in0=girz[:, off:off + n],
                            in1=wt[0:128, 0:n], op=OP.add),
                        nmax=1024)
                rz = wp.tile([P, NPC], f32, name=f"rz{d}", tag="gru_rz")
                nc.scalar.activation(rz[:], rzs[:], AF.Sigmoid, bias=brz[d][:, 0:1])

                gin = wp.tile([64, NPC], f32, name=f"gin{d}", tag="gru_gin")
                psum_mm(wihT[d][:, 128:192], lambda off, n: xc16[:, off:off + n], 64, NPC,
                        lambda wt, off, n: nc.scalar.activation(
                            gin[:, off:off + n], wt[0:64, 0:n], AF.Identity,
                            bias=bihn[d][:, 0:1]),
                        nmax=1024)
                hn = wp.tile([64, NPC], f32, name=f"hn{d}", tag="gru_hn")
                psum_mm(whhT[d][:, 128:192], lambda off, n: h16[:, off:off + n], 64, NPC,
                        lambda wt, off, n: nc.scalar.activation(
                            hn[:, off:off + n], wt[0:64, 0:n], AF.Identity,
                            bias=bhhn[d][:, 0:1]),
                        nmax=1024)
                h_T, h16 = gru_elem(rz, gin, hn, h_T, f"d{d}")

                if d < DEPTHS - 1:
                    h_out(h16, d + 1, f"d{d}")
                    stage_wet(d + 1)
                else:
                    hnm = wp.tile([P, 8 * DIM], bf16, name="hnm_last", tag="hnm")
                    pooled_ps = pmsg.tile([64, 1024], f32, name="pooled_ps", tag="msgT")
                    for c in range(8):
                        nc.scalar.dma_start_transpose(
                            hnm[:, c * DIM:(c + 1) * DIM], h16[:, c * P:(c + 1) * P])
                        nc.tensor.matmul(
                            pooled_ps[0:64, 0:N_GRAPHS],
                            hnm[:, c * DIM:(c + 1) * DIM],
                            pS[:, c * N_GRAPHS:(c + 1) * N_GRAPHS],
                            start=(c == 0), stop=(c == 7),
                        )
                    pooled_sb = wp.tile([64, N_GRAPHS], f32, name="pooled_sb")
                    nc.scalar.activation(pooled_sb[:], pooled_ps[0:64, 0:N_GRAPHS], AF.Copy)
                    nc.sync.dma_start(ar_in[:, :], pooled_sb[:])

            # ---------------- pooling AllReduce + output MLP ----------------
            nc.gpsimd.collective_compute(
                "AllReduce", OP.add, replica_groups=RG,
                ins=[ar_in.opt()], outs=[ar_out.opt()],
            )
            pooled = wp.tile([64, N_GRAPHS], f32, name="pooled")
            nc.sync.dma_start(pooled[:], ar_out[:, :])

            m1_ps = pwe.tile([P, 1024], f32, name="m1_ps", tag="mm")
            nc.tensor.matmul(m1_ps[0:64, 0:N_GRAPHS], o0wT[:], pooled[:],
                             start=True, stop=True)
            m1r = wp.tile([64, N_GRAPHS], f32, name="m1r")
            nc.scalar.activation(m1r[:], m1_ps[0:64, 0:N_GRAPHS], AF.Relu, bias=o0b[:, 0:1])

            m2_ps = pwe.tile([P, 1024], f32, name="m2_ps", tag="mm")
            nc.tensor.matmul(m2_ps[0:32, 0:N_GRAPHS], o1wT[:], m1r[:],
                             start=True, stop=True)
            m2b = wp.tile([32, N_GRAPHS], f32, name="m2b")
            nc.scalar.activation(m2b[:], m2_ps[0:32, 0:N_GRAPHS], AF.Identity,
                                 bias=o1b[:, 0:1])

            m3_ps = pwe.tile([P, 1024], f32, name="m3_ps", tag="mm")
            nc.tensor.matmul(m3_ps[0:1, 0:N_GRAPHS], o2wT[:], m2b[:],
                             start=True, stop=True)
            ysb = wp.tile([1, N_GRAPHS], f32, name="ysb")
            nc.scalar.activation(ysb[:], m3_ps[0:1, 0:N_GRAPHS], AF.Identity,
                                 bias=o2b[:, 0:1])
            nc.sync.dma_start(y_d[:, :], ysb[:])

    nc.finalize()
    return nc


def _prep(inputs):
    import ml_dtypes
    bf16 = ml_dtypes.bfloat16
    g = lambda k: np.asarray(inputs[k])
    x = g("x").astype(np.float32)
    ea = g("edge_attr").astype(np.float32)
    ei = g("edge_index").astype(np.int64)
    batch = g("batch").astype(np.int64)
    src, dst = ei[0], ei[1]

    owner = dst // NPC
    core_ids = [np.nonzero(owner == c)[0] for c in range(NC)]
    T = int(max((len(ids) + P - 1) // P for ids in core_ids))
    T = max(T, 2)
    T += T % 2
    EP = T * P

    cnt = np.bincount(batch, minlength=N_GRAPHS).astype(np.float32)
    inv = 1.0 / np.maximum(cnt, 1.0)

    mlp2_b = g("mlp2_b").astype(np.float32)
    b2_zero = bool(np.all(mlp2_b == 0))

    rmask = np.zeros((P, 32, 64), np.float32)
    pidx = np.arange(P)
    for c in range(32):
        rmask[pidx, c, 2 * c + (pidx >= 64)] = 1.0
    rmask = rmask.reshape(P, 32 * 64)

    shared = {
        "fc0_wT": g("fc0_w").astype(np.float32).T.copy(),
        "fc0_b": g("fc0_b").astype(np.float32)[:, None],
        "g0_wihT": g("gru0_wih").astype(np.float32).T.copy(),
        "g0_brz": (g("gru0_bih") + g("gru0_bhh")).astype(np.float32)[:128, None],
        "g0_bihn": g("gru0_bih").astype(np.float32)[128:, None],
        "g0_bhhn": g("gru0_bhh").astype(np.float32)[128:, None],
        "o0wT": g("out0_w").astype(np.float32).T.copy(),
        "o0b": g("out0_b").astype(np.float32)[:, None],
        "o1wT": g("out1_w").astype(np.float32).T.copy(),
        "o1b": g("out1_b").astype(np.float32)[:, None],
        "o2wT": g("out2_w").astype(np.float32).T.copy(),
        "o2b": g("out2_b").astype(np.float32)[:, None],
        "rmask": rmask.astype(bf16),
    }
    mlp1_w = g("mlp1_w").astype(np.float32)
    mlp1_b = g("mlp1_b").astype(np.float32)
    mlp2_w = g("mlp2_w").astype(np.float32)
    root_w = g("root_w").astype(np.float32)
    conv_b = g("conv_b").astype(np.float32)
    gru_wih = g("gru_wih").astype(np.float32)
    gru_whh = g("gru_whh").astype(np.float32)
    gru_bih = g("gru_bih").astype(np.float32)
    gru_bhh = g("gru_bhh").astype(np.float32)
    for d in range(DEPTHS):
        shared[f"w2p{d}"] = (
            mlp2_w[d].reshape(64, 64, 128).transpose(2, 1, 0).reshape(128, 4096)
        ).astype(bf16)
        shared[f"m1wT{d}"] = mlp1_w[d].T.astype(bf16).copy()
        shared[f"m1b{d}"] = mlp1_b[d][:, None].copy()
        shared[f"root{d}"] = root_w[d].astype(bf16).copy()
        shared[f"convb{d}"] = conv_b[d][:, None].copy()
        shared[f"wihT{d}"] = gru_wih[d].T.astype(bf16).copy()
        shared[f"whhT{d}"] = gru_whh[d].T.astype(bf16).copy()
        shared[f"brz{d}"] = (gru_bih[d] + gru_bhh[d])[:128, None].copy()
        shared[f"bihn{d}"] = gru_bih[d][128:, None].copy()
        shared[f"bhhn{d}"] = gru_bhh[d][128:, None].copy()
        if not b2_zero:
            io = np.arange(4096)
            o, i = io // 64, io % 64
            b2t = mlp2_b[d][i * 64 + o].reshape(32, 128).T.copy()
            shared[f"b2t{d}"] = b2t.astype(np.float32)

    in_maps = []
    for c in range(NC):
        ids = core_ids[c]
        n_real = len(ids)
        src_pad = np.zeros(EP, np.int32)
        src_pad[:n_real] = src[ids]
        dst_pad = np.full(EP, 2047, np.float32)
        dst_pad[:n_real] = (dst[ids] - c * NPC).astype(np.float32)
        ea_pad = np.zeros((EP, 10), np.float32)
        ea_pad[:n_real] = ea[ids]
        pm = np.zeros((NPC, N_GRAPHS), np.float32)
        nb = batch[c * NPC:(c + 1) * NPC]
        pm[np.arange(NPC), nb] = inv[nb]
        m = {
            "xT": x[c * NPC:(c + 1) * NPC].T.copy(),
            "eaT": ea_pad.T.astype(bf16).copy(),
            "srcidx": src_pad.reshape(T, P).T.copy(),
            "dsti": dst_pad.reshape(T, P).T.copy(),
            "poolS": pm.astype(bf16),
        }
        m.update(shared)
        in_maps.append(m)
    return T, b2_zero, in_maps


def kernel(**inputs) -> np.ndarray:
    global LAST_EXEC_NS, LAST_RESULTS
    T, b2_zero, in_maps = _prep(inputs)
    key = (T, b2_zero)
    if key not in _CACHE:
        _CACHE[key] = _build(T, b2_zero)
    nc = _CACHE[key]

    from concourse.bass_utils import run_bass_kernel_spmd

    if TRACE:
        res = run_bass_kernel_spmd(
            nc, in_maps, list(range(NC)), trace=True, trace_cores=list(range(NC))
        )
        LAST_EXEC_NS = res.exec_time_ns
        LAST_RESULTS = res
    else:
        res = run_bass_kernel_spmd(nc, in_maps, list(range(NC)))
    return res.results[0]["y"].reshape(N_GRAPHS).astype(np.float32)


# revision 12
# speedup vs baseline: 1.0227x; 1.0227x over previous
"""NNConv+GRU message-passing network (ConvGRU) on 8 Trainium2 NeuronCores.

v2: mixed-path per-edge contraction, balancing PE / ACT / DVE / GPSIMD:

  msg[e,o] = sum_i h[src[e],i] * We[e,i,o],  We = edge-MLP(edge_attr)

  - BASE path (oi-chunks q < Q_BASE, edge-major): We tile [e128, 1024] = PE
    (hidT-tile stationary), ACT evacuates PSUM->bf16, DVE (or GPSIMD)
    broadcast-multiplies by gathered h, DVE tensor_reduce sums i.
  - V1 path (chunks c >= 8*Q_BASE, (o,i)-major): WeT chunk [(o2,i64), e] = PE
    (w2p chunk stationary), PSUM evacuated by ACT (or read directly by DVE),
    multiplied by the xbar-transposed+duplicated gathered h, then reduced
    over i by a PE mask-matmul into msgT rows; msgT is xbar-transposed back
    into the edge-major msg tiles.  KS chunks are staged (MM+evac) into SBUF
    during the previous AllGather window.

  Scatter-add to nodes = per-tile selection matmul (S built on-chip from dst
  indices).  Everything bf16 except GRU state math and pooling accumulation.
"""
import numpy as np

DIM = 64
DEPTHS = 3
N_NODES = 8192
N_EDGES = 16384
N_GRAPHS = 64
NC = 8
NPC = N_NODES // NC
P = 128

Q_BASE = 2        # oi-1024-chunks on the BASE path (edge-major, DVE reduce)
KS = 5            # staged V1 chunks (MM+evac pre-run during AllGather window)
RMM_LAG = 2       # software pipelining for the V1 mask-matmul
GPS_EVERY = 3     # every GPS_EVERY-th base (t,q) multiply goes to GPSIMD
DIR_EVERY = 3     # every DIR_EVERY-th in-loop V1 chunk multiplies from PSUM

TRACE = False
LAST_EXEC_NS = None
LAST_RESULTS = None

_CACHE = {}


def _build(T, b2_zero):
    import concourse.mybir as mybir
    import concourse.tile as tile
    from concourse import bacc
    import concourse.bass as bass
    from concourse.masks import make_identity

    f32 = mybir.dt.float32
    bf16 = mybir.dt.bfloat16
    i32 = mybir.dt.int32
    AF = mybir.ActivationFunctionType
    OP = mybir.AluOpType
    EP = T * P

    q_base = Q_BASE if b2_zero else 0  # base path has no bias support
    v1_c0 = 8 * q_base
    ks = min(KS, 32 - v1_c0)

    echunks = []
    t0 = 0
    while t0 < T:
        nt = min(8, T - t0)
        echunks.append((t0, nt))
        t0 += nt

    nc = bacc.Bacc("TRN2", target_bir_lowering=False, debug=False, num_devices=NC)

    def din(name, shape, dt=f32):
        return nc.dram_tensor(name, shape, dt, kind="ExternalInput")

    xT_d = din("xT", [40, NPC])
    eaT_d = din("eaT", [10, EP], bf16)
    srcx_d = din("srcidx", [P, T], i32)
    dsti_d = din("dsti", [P, T])
    rmask_d = din("rmask", [P, 32 * 64], bf16)
    pS_d = din("poolS", [NPC, N_GRAPHS], bf16)
    fc0_wT_d = din("fc0_wT", [40, 32])
    fc0_b_d = din("fc0_b", [32, 1])
    g0_wihT_d = din("g0_wihT", [32, 192])
    g0_brz_d = din("g0_brz", [128, 1])
    g0_bihn_d = din("g0_bihn", [64, 1])
    g0_bhhn_d = din("g0_bhhn", [64, 1])
    w2p_d = [din(f"w2p{d}", [128, 4096], bf16) for d in range(DEPTHS)]
    m1wT_d = [din(f"m1wT{d}", [10, 128], bf16) for d in range(DEPTHS)]
    m1b_d = [din(f"m1b{d}", [128, 1]) for d in range(DEPTHS)]
    root_d = [din(f"root{d}", [64, 64], bf16) for d in range(DEPTHS)]
    convb_d = [din(f"convb{d}", [64, 1]) for d in range(DEPTHS)]
    wihT_d = [din(f"wihT{d}", [64, 192], bf16) for d in range(DEPTHS)]
    whhT_d = [din(f"whhT{d}", [64, 192], bf16) for d in range(DEPTHS)]
    brz_d = [din(f"brz{d}", [128, 1]) for d in range(DEPTHS)]
    bihn_d = [din(f"bihn{d}", [64, 1]) for d in range(DEPTHS)]
    bhhn_d = [din(f"bhhn{d}", [64, 1]) for d in range(DEPTHS)]
    b2t_d = None if b2_zero else [din(f"b2t{d}", [128, 32]) for d in range(DEPTHS)]
    o0wT_d = din("o0wT", [64, 64])
    o0b_d = din("o0b", [64, 1])
    o1wT_d = din("o1wT", [64, 32])
    o1b_d = din("o1b", [32, 1])
    o2wT_d = din("o2wT", [32, 1])
    o2b_d = din("o2b", [1, 1])

    y_d = nc.dram_tensor("y", [1, N_GRAPHS], f32, kind="ExternalOutput")

    RG = [list(range(NC))]

    with tile.TileContext(nc) as tc:
        with (
            tc.tile_pool(name="const", bufs=1) as cp,
            tc.tile_pool(name="work", bufs=1) as wp,
            tc.tile_pool(name="edge", bufs=6) as ep,
            tc.tile_pool(name="prod", bufs=3) as pp,
            tc.tile_pool(name="msgp", bufs=12) as mp,
            tc.tile_pool(name="weps", bufs=2, space="PSUM") as pwe,
            tc.tile_pool(name="pmsg", bufs=1, space="PSUM") as pmsg,
            tc.tile_pool(name="pagg", bufs=1, space="PSUM") as pagg,
            tc.tile_pool(name="dram", bufs=1, space="DRAM") as dp,
        ):
            def load(name, dram, shape, dt=f32, eng=None):
                t = cp.tile(shape, dt, name=name)
                (eng or nc.sync).dma_start(t[:], dram[:, :])
                return t

            # phase0-critical loads on the scalar queue, bulk on gpsimd
            xT = load("xT_s", xT_d, [40, NPC], eng=nc.scalar)
            fc0_wT = load("fc0_wT_s", fc0_wT_d, [40, 32], eng=nc.scalar)
            fc0_b = load("fc0_b_s", fc0_b_d, [32, 1], eng=nc.scalar)
            g0_wihT = load("g0_wihT_s", g0_wihT_d, [32, 192], eng=nc.scalar)
            g0_brz = load("g0_brz_s", g0_brz_d, [128, 1], eng=nc.scalar)
            g0_bihn = load("g0_bihn_s", g0_bihn_d, [64, 1], eng=nc.scalar)
            g0_bhhn = load("g0_bhhn_s", g0_bhhn_d, [64, 1], eng=nc.scalar)
            srcx = load("srcx_s", srcx_d, [P, T], i32, eng=nc.scalar)
            dsti = load("dsti_s", dsti_d, [P, T], eng=nc.scalar)
            eaT = load("eaT_s", eaT_d, [10, EP], bf16, eng=nc.scalar)
            m1wT = [load(f"m1wT_s{d}", m1wT_d[d], [10, 128], bf16, eng=nc.scalar)
                    for d in range(DEPTHS)]
            m1b = [load(f"m1b_s{d}", m1b_d[d], [128, 1], eng=nc.scalar)
                   for d in range(DEPTHS)]
            rmask = load("rmask_s", rmask_d, [P, 32 * 64], bf16)
            w2p = [load(f"w2p_s{d}", w2p_d[d], [128, 4096], bf16) for d in range(DEPTHS)]
            rootw = [load(f"root_s{d}", root_d[d], [64, 64], bf16) for d in range(DEPTHS)]
            convb = [load(f"convb_s{d}", convb_d[d], [64, 1]) for d in range(DEPTHS)]
            wihT = [load(f"wihT_s{d}", wihT_d[d], [64, 192], bf16) for d in range(DEPTHS)]
            whhT = [load(f"whhT_s{d}", whhT_d[d], [64, 192], bf16) for d in range(DEPTHS)]
            brz = [load(f"brz_s{d}", brz_d[d], [128, 1]) for d in range(DEPTHS)]
            bihn = [load(f"bihn_s{d}", bihn_d[d], [64, 1]) for d in range(DEPTHS)]
            bhhn = [load(f"bhhn_s{d}", bhhn_d[d], [64, 1]) for d in range(DEPTHS)]
            b2t = (
                None if b2_zero else
                [load(f"b2t_s{d}", b2t_d[d], [128, 32]) for d in range(DEPTHS)]
            )
            o0wT = load("o0wT_s", o0wT_d, [64, 64])
            o0b = load("o0b_s", o0b_d, [64, 1])
            o1wT = load("o1wT_s", o1wT_d, [64, 32])
            o1b = load("o1b_s", o1b_d, [32, 1])
            o2wT = load("o2wT_s", o2wT_d, [32, 1])
            o2b = load("o2b_s", o2b_d, [1, 1])
            pS = cp.tile([P, 8 * N_GRAPHS], bf16, name="pS_s")
            for c in range(8):
                nc.sync.dma_start(
                    pS[:, c * N_GRAPHS:(c + 1) * N_GRAPHS],
                    pS_d[c * P:(c + 1) * P, :],
                )

            # S selection matrix on-chip: S[e, t*NPC+n] = (dsti[e,t]==n)
            iot = cp.tile([P, NPC], f32, name="iot")
            nc.gpsimd.iota(iot[:], pattern=[[1, NPC]], base=0, channel_multiplier=0,
                           allow_small_or_imprecise_dtypes=True)
            S = cp.tile([P, T * NPC], bf16, name="S_s")
            for t in range(T):
                nc.vector.tensor_scalar(
                    out=S[:, t * NPC:(t + 1) * NPC], in0=iot[:],
                    scalar1=dsti[:, t:t + 1], scalar2=None, op0=OP.is_equal,
                )

            hown = [dp.tile([NPC, DIM], bf16, name=f"hown{d}") for d in range(DEPTHS)]
            hfull = [dp.tile([N_NODES, DIM], bf16, name=f"hfull{d}") for d in range(DEPTHS)]
            ar_in = dp.tile([DIM, N_GRAPHS], f32, name="ar_in")
            ar_out = dp.tile([DIM, N_GRAPHS], f32, name="ar_out")

            hidT = [cp.tile([P, EP], bf16, name=f"hidT{d}") for d in range(DEPTHS)]
            westg = [cp.tile([P, EP], bf16, name=f"westg{c}") for c in range(ks)]

            _nctr = [0]

            def psum_mm(lhsT, rhs_fn, m, n_total, out_fn, nmax=512):
                off = 0
                while off < n_total:
                    n = min(nmax, n_total - off)
                    _nctr[0] += 1
                    w = pwe.tile([P, 1024], f32, name=f"w{_nctr[0]}", tag="mm")
                    if n > 512:
                        nc.tensor.matmul(w[0:m, 0:512], lhsT, rhs_fn(off, 512),
                                         start=True, stop=True)
                        nc.tensor.matmul(w[0:m, 512:n], lhsT, rhs_fn(off + 512, n - 512),
                                         start=True, stop=True)
                    else:
                        nc.tensor.matmul(w[0:m, 0:n], lhsT, rhs_fn(off, n),
                                         start=True, stop=True)
                    out_fn(w, off, n)
                    off += n

            def gru_elem(rz_s, gi_n_s, hn_s, h_prev, tagp):
                z_s = wp.tile([64, NPC], f32, name=f"z_{tagp}", tag="gru_z")
                nc.sync.dma_start(z_s[:], rz_s[64:128, :])
                t1 = wp.tile([64, NPC], f32, name=f"t1_{tagp}", tag="gru_t1")
                nc.vector.tensor_tensor(out=t1[:], in0=rz_s[0:64, :], in1=hn_s[:], op=OP.mult)
                nc.vector.tensor_tensor(out=t1[:], in0=t1[:], in1=gi_n_s[:], op=OP.add)
                nt = wp.tile([64, NPC], f32, name=f"nt_{tagp}", tag="gru_nt")
                nc.scalar.activation(nt[:], t1[:], AF.Tanh)
                hm = wp.tile([64, NPC], f32, name=f"hm_{tagp}", tag="gru_hm")
                if h_prev is None:
                    nc.vector.tensor_tensor(out=hm[:], in0=z_s[:], in1=nt[:], op=OP.mult)
                    hnew = wp.tile([64, NPC], f32, name=f"h_{tagp}", tag="hT")
                    nc.vector.tensor_tensor(out=hnew[:], in0=nt[:], in1=hm[:], op=OP.subtract)
                else:
                    nc.vector.tensor_tensor(out=hm[:], in0=h_prev[:], in1=nt[:], op=OP.subtract)
                    nc.vector.tensor_tensor(out=hm[:], in0=hm[:], in1=z_s[:], op=OP.mult)
                    hnew = wp.tile([64, NPC], f32, name=f"h_{tagp}", tag="hT")
                    nc.vector.tensor_tensor(out=hnew[:], in0=hm[:], in1=nt[:], op=OP.add)
                h16 = wp.tile([64, NPC], bf16, name=f"h16_{tagp}", tag="hT16")
                nc.vector.tensor_copy(h16[:], hnew[:])
                return hnew, h16

            def h_out(h16, d_next, tagp):
                hnm = wp.tile([P, 8 * DIM], bf16, name=f"hnm_{tagp}", tag="hnm")
                for c in range(8):
                    nc.sync.dma_start_transpose(
                        hnm[:, c * DIM:(c + 1) * DIM], h16[:, c * P:(c + 1) * P]
                    )
                    nc.sync.dma_start(
                        hown[d_next][c * P:(c + 1) * P, :],
                        hnm[:, c * DIM:(c + 1) * DIM],
                    )
                nc.gpsimd.collective_compute(
                    "AllGather", OP.bypass, replica_groups=RG,
                    ins=[hown[d_next].opt()], outs=[hfull[d_next].opt()],
                )

            def stage_wet(d):
                for j in range(ks):
                    c = v1_c0 + j
                    def evac(wt, off, n, _c=c, _j=j):
                        if b2_zero:
                            nc.scalar.activation(
                                westg[_j][:, off:off + n], wt[:, 0:n], AF.Copy)
                        else:
                            nc.scalar.activation(
                                westg[_j][:, off:off + n], wt[:, 0:n],
                                AF.Identity, bias=b2t[d][:, _c:_c + 1])
                    psum_mm(w2p[d][:, c * P:(c + 1) * P],
                            lambda off, n: hidT[d][:, off:off + n],
                            P, EP, evac, nmax=1024)

            def build_hid(d):
                def evac(wt, off, n, _d=d):
                    nc.scalar.activation(
                        hidT[_d][:, off:off + n], wt[:, 0:n], AF.Relu,
                        bias=m1b[_d][:, 0:1])
                psum_mm(m1wT[d][:], lambda off, n, _d=d: eaT[:, off:off + n],
                        P, EP, evac, nmax=1024)

            # ---------------- phase 0 ----------------
            x0r = wp.tile([32, NPC], f32, name="x0r")
            psum_mm(fc0_wT[:], lambda off, n: xT[:, off:off + n], 32, NPC,
                    lambda wt, off, n: nc.scalar.activation(
                        x0r[:, off:off + n], wt[0:32, 0:n], AF.Relu,
                        bias=fc0_b[:, 0:1]))

            rz0 = wp.tile([P, NPC], f32, name="rz0", tag="gru_rz")
            psum_mm(g0_wihT[:, 0:128], lambda off, n: x0r[:, off:off + n], P, NPC,
                    lambda wt, off, n: nc.scalar.activation(
                        rz0[:, off:off + n], wt[0:128, 0:n], AF.Sigmoid,
                        bias=g0_brz[:, 0:1]))

            gin0 = wp.tile([64, NPC], f32, name="gin0", tag="gru_gin")
            psum_mm(g0_wihT[:, 128:192], lambda off, n: x0r[:, off:off + n], 64, NPC,
                    lambda wt, off, n: nc.scalar.activation(
                        gin0[:, off:off + n], wt[0:64, 0:n], AF.Identity,
                        bias=g0_bihn[:, 0:1]))
            hn0 = wp.tile([64, NPC], f32, name="hn0", tag="gru_hn")
            nc.vector.tensor_scalar_mul(hn0[:], rz0[0:64, :], g0_bhhn[:, 0:1])
            h_T, h16 = gru_elem(rz0, gin0, hn0, None, "p0")
            h_out(h16, 0, "p0")

            for d in range(DEPTHS):
                build_hid(d)
            stage_wet(0)

            # ---------------- conv depths ----------------
            for d in range(DEPTHS):
                # --- phase A: gathers (+ transposed copy for the V1 path) ---
                hsrcT2 = wp.tile([P, EP], bf16, name=f"hsrc{d}", tag="hsrc")
                hsf2s = []
                for pr in range(T // 2):
                    t0, t1 = 2 * pr, 2 * pr + 1
                    hsf2 = ep.tile([P, P], bf16, name=f"hsf{d}_{pr}", tag="hsf", bufs=10)
                    for j, t in enumerate((t0, t1)):
                        nc.gpsimd.indirect_dma_start(
                            out=hsf2[:, j * 64:(j + 1) * 64], out_offset=None,
                            in_=hfull[d][:, :],
                            in_offset=bass.IndirectOffsetOnAxis(
                                ap=srcx[:, t:t + 1], axis=0),
                        )
                    hsf2s.append(hsf2)
                    tmpT = ep.tile([P, P], bf16, name=f"tmpT{d}_{pr}", tag="tmpT")
                    nc.sync.dma_start_transpose(tmpT[:], hsf2[:])
                    for j, t in enumerate((t0, t1)):
                        nc.sync.dma_start(
                            hsrcT2[0:64, t * P:(t + 1) * P], tmpT[j * 64:(j + 1) * 64, :])
                        nc.sync.dma_start(
                            hsrcT2[64:128, t * P:(t + 1) * P], tmpT[j * 64:(j + 1) * 64, :])

                aggT = pagg.tile([64, NPC], f32, name=f"aggT{d}", tag="agg")
                n_sc = [0]

                def scatter_tile(t, msg_t):
                    for s in range(2):
                        nc.tensor.matmul(
                            aggT[0:64, s * 512:(s + 1) * 512],
                            msg_t[:],
                            S[:, t * NPC + s * 512: t * NPC + (s + 1) * 512],
                            start=(n_sc[0] == 0), stop=False,
                        )
                    n_sc[0] += 1

                pend_rmm = []
                pend_scatter = []

                def flush_rmm():
                    msgT_, c_, pt_, cols_ = pend_rmm.pop(0)
                    off = 0
                    while off < cols_:
                        n = min(512, cols_ - off)
                        nc.tensor.matmul(
                            msgT_[0:64, off:off + n],
                            rmask[:, c_ * 64:(c_ + 1) * 64], pt_[:, off:off + n],
                            start=(c_ == v1_c0), stop=(c_ == 31))
                        off += n

                gps_ctr = [0]
                for ci, (tc0, ntc) in enumerate(echunks):
                    cols = ntc * P
                    ec0 = tc0 * P
                    msgtiles = [mp.tile([P, DIM], bf16, name=f"msg{d}_{tc0 + tt}",
                                        tag="msg") for tt in range(ntc)]

                    # ---- BASE path: per (tile, q) edge-major ----
                    for tt in range(ntc):
                        t = tc0 + tt
                        hsb = hsf2s[t // 2][:, (t % 2) * 64:(t % 2 + 1) * 64]
                        msgf = wp.tile([P, 16 * q_base], f32, name=f"msgf{d}_{t}",
                                       tag="msgf", bufs=4) if q_base else None
                        for q in range(q_base):
                            _nctr[0] += 1
                            w = pwe.tile([P, 1024], f32, name=f"bw{_nctr[0]}", tag="mm")
                            for s in range(2):
                                nc.tensor.matmul(
                                    w[:, s * 512:(s + 1) * 512],
                                    hidT[d][:, t * P:(t + 1) * P],
                                    w2p[d][:, q * 1024 + s * 512:q * 1024 + (s + 1) * 512],
                                    start=True, stop=True)
                            wsb = pp.tile([P, 1024], bf16, name=f"bwsb{d}_{t}_{q}",
                                          tag="wsb", bufs=3)
                            nc.scalar.activation(wsb[:], w[:], AF.Copy)
                            prod = pp.tile([P, 1024], bf16, name=f"bprod{d}_{t}_{q}",
                                           tag="prod", bufs=3)
                            gps_ctr[0] += 1
                            veng = nc.gpsimd if gps_ctr[0] % GPS_EVERY == 0 else nc.vector
                            veng.tensor_tensor(
                                out=prod[:].rearrange("p (o i) -> p o i", i=64),
                                in0=wsb[:].rearrange("p (o i) -> p o i", i=64),
                                in1=hsb.unsqueeze(1).to_broadcast([P, 16, 64]),
                                op=OP.mult)
                            nc.vector.tensor_reduce(
                                out=msgf[:, q * 16:(q + 1) * 16],
                                in_=prod[:].rearrange("p (o i) -> p o i", i=64),
                                axis=mybir.AxisListType.X, op=OP.add)
                        if q_base:
                            nc.vector.tensor_copy(
                                msgtiles[tt][:, 0:16 * q_base], msgf[:])

                    # ---- V1 path ----
                    msgT = pmsg.tile([64, 1024], f32, name=f"msgT{d}_{ci}", tag="msgT")
                    for c in range(v1_c0, 32):
                        j = c - v1_c0
                        pt = pp.tile([P, 1024], bf16, name=f"pt{d}_{ci}_{c}",
                                     tag="pt", bufs=4)
                        if j < ks:
                            nc.vector.tensor_tensor(
                                out=pt[:, 0:cols],
                                in0=westg[j][:, ec0:ec0 + cols],
                                in1=hsrcT2[:, ec0:ec0 + cols], op=OP.mult)
                        else:
                            direct = ((j - ks) % DIR_EVERY == 1)
                            off = 0
                            while off < cols:
                                n = min(512, cols - off)
                                _nctr[0] += 1
                                w = pwe.tile([P, 1024], f32, name=f"vw{_nctr[0]}",
                                             tag="mm")
                                nc.tensor.matmul(
                                    w[:, 0:n],
                                    w2p[d][:, c * P:(c + 1) * P],
                                    hidT[d][:, ec0 + off:ec0 + off + n],
                                    start=True, stop=True)
                                if direct and b2_zero:
                                    nc.vector.tensor_tensor(
                                        out=pt[:, off:off + n], in0=w[:, 0:n],
                                        in1=hsrcT2[:, ec0 + off:ec0 + off + n],
                                        op=OP.mult)
                                elif direct:
                                    nc.vector.scalar_tensor_tensor(
                                        out=pt[:, off:off + n], in0=w[:, 0:n],
                                        scalar=b2t[d][:, c:c + 1],
                                        in1=hsrcT2[:, ec0 + off:ec0 + off + n],
                                        op0=OP.add, op1=OP.mult)
                                else:
                                    wsb = pp.tile([P, 1024], bf16,
                                                  name=f"vwsb{d}_{ci}_{c}_{off}",
                                                  tag="wsb", bufs=3)
                                    if b2_zero:
                                        nc.scalar.activation(
                                            wsb[:, 0:n], w[:, 0:n], AF.Copy)
                                    else:
                                        nc.scalar.activation(
                                            wsb[:, 0:n], w[:, 0:n], AF.Identity,
                                            bias=b2t[d][:, c:c + 1])
                                    nc.vector.tensor_tensor(
                                        out=pt[:, off:off + n], in0=wsb[:, 0:n],
                                        in1=hsrcT2[:, ec0 + off:ec0 + off + n],
                                        op=OP.mult)
                                off += n
                        pend_rmm.append((msgT, c, pt, cols))
                        if len(pend_rmm) > RMM_LAG:
                            flush_rmm()
                        if c == v1_c0 + 6 and pend_scatter:
                            while pend_scatter:
                                scatter_tile(*pend_scatter.pop(0))
                    while pend_rmm:
                        flush_rmm()

                    # evacuate V1 msgT rows and transpose into msg tiles
                    ob = 2 * v1_c0   # first V1 o-column
                    msgS = wp.tile([64 - ob, 1024], bf16, name=f"msgS{d}_{ci}",
                                   tag="msgS", bufs=2)
                    nc.scalar.activation(msgS[:, 0:cols], msgT[ob:64, 0:cols], AF.Copy)
                    for tt in range(ntc):
                        nc.sync.dma_start_transpose(
                            msgtiles[tt][:, ob:64], msgS[:, tt * P:(tt + 1) * P])
                        pend_scatter.append((tc0 + tt, msgtiles[tt]))
                while pend_scatter:
                    scatter_tile(*pend_scatter.pop(0))

                # root contribution + bias + relu
                for s in range(2):
                    nc.tensor.matmul(
                        aggT[0:64, s * 512:(s + 1) * 512],
                        rootw[d][:],
                        h16[:, s * 512:(s + 1) * 512],
                        start=False, stop=(s == 1),
                    )
                xc = wp.tile([64, NPC], f32, name=f"xc{d}", tag="xc")
                nc.scalar.activation(xc[:], aggT[0:64, :], AF.Relu, bias=convb[d][:, 0:1])
                xc16 = wp.tile([64, NPC], bf16, name=f"xc16{d}", tag="xc16")
                nc.vector.tensor_copy(xc16[:], xc[:])

                # ---- GRU(xc, h) ----
                girz = wp.tile([P, NPC], f32, name=f"girz{d}", tag="gru_girz")
                psum_mm(wihT[d][:, 0:128], lambda off, n: xc16[:, off:off + n], P, NPC,
                        lambda wt, off, n: nc.scalar.activation(
                            girz[:, off:off + n], wt[0:128, 0:n], AF.Copy),
                        nmax=1024)
                rzs = wp.tile([P, NPC], f32, name=f"rzs{d}", tag="gru_rzs")
                psum_mm(whhT[d][:, 0:128], lambda off, n: h16[:, off:off + n], P, NPC,
                        lambda wt, off, n: nc.vector.tensor_tensor(
                            out=rzs[:, off:off + n], in0=girz[:, off:off + n],
                            in1=wt[0:128, 0:n], op=OP.add),
                        nmax=1024)
                rz = wp.tile([P, NPC], f32, name=f"rz{d}", tag="gru_rz")
                nc.scalar.activation(rz[:], rzs[:], AF.Sigmoid, bias=brz[d][:, 0:1])

                gin = wp.tile([64, NPC], f32, name=f"gin{d}", tag="gru_gin")
                psum_mm(wihT[d][:, 128:192], lambda off, n: xc16[:, off:off + n], 64, NPC,
                        lambda wt, off, n: nc.scalar.activation(
                            gin[:, off:off + n], wt[0:64, 0:n], AF.Identity,
                            bias=bihn[d][:, 0:1]),
                        nmax=1024)
                hn = wp.tile([64, NPC], f32, name=f"hn{d}", tag="gru_hn")
                psum_mm(whhT[d][:, 128:192], lambda off, n: h16[:, off:off + n], 64, NPC,
                        lambda wt, off, n: nc.scalar.activation(
                            hn[:, off:off + n], wt[0:64, 0:n], AF.Identity,
                            bias=bhhn[d][:, 0:1]),
                        nmax=1024)
                h_T, h16 = gru_elem(rz, gin, hn, h_T, f"d{d}")

                if d < DEPTHS - 1:
                    h_out(h16, d + 1, f"d{d}")
                    stage_wet(d + 1)
                else:
                    hnm = wp.tile([P, 8 * DIM], bf16, name="hnm_last", tag="hnm")
                    pooled_ps = pmsg.tile([64, 1024], f32, name="pooled_ps", tag="msgT")
                    for c in range(8):
                        nc.sync.dma_start_transpose(
                            hnm[:, c * DIM:(c + 1) * DIM], h16[:, c * P:(c + 1) * P])
                        nc.tensor.matmul(
                            pooled_ps[0:64, 0:N_GRAPHS],
                            hnm[:, c * DIM:(c + 1) * DIM],
                            pS[:, c * N_GRAPHS:(c + 1) * N_GRAPHS],
                            start=(c == 0), stop=(c == 7),
                        )
                    pooled_sb = wp.tile([64, N_GRAPHS], f32, name="pooled_sb")
                    nc.scalar.activation(pooled_sb[:], pooled_ps[0:64, 0:N_GRAPHS], AF.Copy)
                    nc.sync.dma_start(ar_in[:, :], pooled_sb[:])

            # ---------------- pooling AllReduce + output MLP ----------------
            nc.gpsimd.collective_compute(
                "AllReduce", OP.add, replica_groups=RG,
                ins=[ar_in.opt()], outs=[ar_out.opt()],
            )
            pooled = wp.tile([64, N_GRAPHS], f32, name="pooled")
            nc.sync.dma_start(pooled[:], ar_out[:, :])

            m1_ps = pwe.tile([P, 1024], f32, name="m1_ps", tag="mm")
            nc.tensor.matmul(m1_ps[0:64, 0:N_GRAPHS], o0wT[:], pooled[:],
                             start=True, stop=True)
            m1r = wp.tile([64, N_GRAPHS], f32, name="m1r")
            nc.scalar.activation(m1r[:], m1_ps[0:64, 0:N_GRAPHS], AF.Relu, bias=o0b[:, 0:1])

            m2_ps = pwe.tile([P, 1024], f32, name="m2_ps", tag="mm")
            nc.tensor.matmul(m2_ps[0:32, 0:N_GRAPHS], o1wT[:], m1r[:],
                             start=True, stop=True)
            m2b = wp.tile([32, N_GRAPHS], f32, name="m2b")
            nc.scalar.activation(m2b[:], m2_ps[0:32, 0:N_GRAPHS], AF.Identity,
                                 bias=o1b[:, 0:1])

            m3_ps = pwe.tile([P, 1024], f32, name="m3_ps", tag="mm")
            nc.tensor.matmul(m3_ps[0:1, 0:N_GRAPHS], o2wT[:], m2b[:],
                             start=True, stop=True)
            ysb = wp.tile([1, N_GRAPHS], f32, name="ysb")
            nc.scalar.activation(ysb[:], m3_ps[0:1, 0:N_GRAPHS], AF.Identity,
                                 bias=o2b[:, 0:1])
            nc.sync.dma_start(y_d[:, :], ysb[:])

    nc.finalize()
    return nc


def _prep(inputs):
    import ml_dtypes
    bf16 = ml_dtypes.bfloat16
    g = lambda k: np.asarray(inputs[k])
    x = g("x").astype(np.float32)
    ea = g("edge_attr").astype(np.float32)
    ei = g("edge_index").astype(np.int64)
    batch = g("batch").astype(np.int64)
    src, dst = ei[0], ei[1]

    owner = dst // NPC
    core_ids = [np.nonzero(owner == c)[0] for c in range(NC)]
    T = int(max((len(ids) + P - 1) // P for ids in core_ids))
    T = max(T, 2)
    T += T % 2
    EP = T * P

    cnt = np.bincount(batch, minlength=N_GRAPHS).astype(np.float32)
    inv = 1.0 / np.maximum(cnt, 1.0)

    mlp2_b = g("mlp2_b").astype(np.float32)
    b2_zero = bool(np.all(mlp2_b == 0))

    rmask = np.zeros((P, 32, 64), np.float32)
    pidx = np.arange(P)
    for c in range(32):
        rmask[pidx, c, 2 * c + (pidx >= 64)] = 1.0
    rmask = rmask.reshape(P, 32 * 64)

    shared = {
        "fc0_wT": g("fc0_w").astype(np.float32).T.copy(),
        "fc0_b": g("fc0_b").astype(np.float32)[:, None],
        "g0_wihT": g("gru0_wih").astype(np.float32).T.copy(),
        "g0_brz": (g("gru0_bih") + g("gru0_bhh")).astype(np.float32)[:128, None],
        "g0_bihn": g("gru0_bih").astype(np.float32)[128:, None],
        "g0_bhhn": g("gru0_bhh").astype(np.float32)[128:, None],
        "o0wT": g("out0_w").astype(np.float32).T.copy(),
        "o0b": g("out0_b").astype(np.float32)[:, None],
        "o1wT": g("out1_w").astype(np.float32).T.copy(),
        "o1b": g("out1_b").astype(np.float32)[:, None],
        "o2wT": g("out2_w").astype(np.float32).T.copy(),
        "o2b": g("out2_b").astype(np.float32)[:, None],
        "rmask": rmask.astype(bf16),
    }
    mlp1_w = g("mlp1_w").astype(np.float32)
    mlp1_b = g("mlp1_b").astype(np.float32)
    mlp2_w = g("mlp2_w").astype(np.float32)
    root_w = g("root_w").astype(np.float32)
    conv_b = g("conv_b").astype(np.float32)
    gru_wih = g("gru_wih").astype(np.float32)
    gru_whh = g("gru_whh").astype(np.float32)
    gru_bih = g("gru_bih").astype(np.float32)
    gru_bhh = g("gru_bhh").astype(np.float32)
    for d in range(DEPTHS):
        shared[f"w2p{d}"] = (
            mlp2_w[d].reshape(64, 64, 128).transpose(2, 1, 0).reshape(128, 4096)
        ).astype(bf16)
        shared[f"m1wT{d}"] = mlp1_w[d].T.astype(bf16).copy()
        shared[f"m1b{d}"] = mlp1_b[d][:, None].copy()
        shared[f"root{d}"] = root_w[d].astype(bf16).copy()
        shared[f"convb{d}"] = conv_b[d][:, None].copy()
        shared[f"wihT{d}"] = gru_wih[d].T.astype(bf16).copy()
        shared[f"whhT{d}"] = gru_whh[d].T.astype(bf16).copy()
        shared[f"brz{d}"] = (gru_bih[d] + gru_bhh[d])[:128, None].copy()
        shared[f"bihn{d}"] = gru_bih[d][128:, None].copy()
        shared[f"bhhn{d}"] = gru_bhh[d][128:, None].copy()
        if not b2_zero:
            io = np.arange(4096)
            o, i = io // 64, io % 64
            b2t = mlp2_b[d][i * 64 + o].reshape(32, 128).T.copy()
            shared[f"b2t{d}"] = b2t.astype(np.float32)

    in_maps = []
    for c in range(NC):
        ids = core_ids[c]
        n_real = len(ids)
        src_pad = np.zeros(EP, np.int32)
        src_pad[:n_real] = src[ids]
        dst_pad = np.full(EP, 2047, np.float32)
        dst_pad[:n_real] = (dst[ids] - c * NPC).astype(np.float32)
        ea_pad = np.zeros((EP, 10), np.float32)
        ea_pad[:n_real] = ea[ids]
        pm = np.zeros((NPC, N_GRAPHS), np.float32)
        nb = batch[c * NPC:(c + 1) * NPC]
        pm[np.arange(NPC), nb] = inv[nb]
        m = {
            "xT": x[c * NPC:(c + 1) * NPC].T.copy(),
            "eaT": ea_pad.T.astype(bf16).copy(),
            "srcidx": src_pad.reshape(T, P).T.copy(),
            "dsti": dst_pad.reshape(T, P).T.copy(),
            "poolS": pm.astype(bf16),
        }
        m.update(shared)
        in_maps.append(m)
    return T, b2_zero, in_maps


def kernel(**inputs) -> np.ndarray:
    global LAST_EXEC_NS, LAST_RESULTS
    T, b2_zero, in_maps = _prep(inputs)
    key = (T, b2_zero)
    if key not in _CACHE:
        _CACHE[key] = _build(T, b2_zero)
    nc = _CACHE[key]

    from concourse.bass_utils import run_bass_kernel_spmd

    if TRACE:
        res = run_bass_kernel_spmd(
            nc, in_maps, list(range(NC)), trace=True, trace_cores=list(range(NC))
        )
        LAST_EXEC_NS = res.exec_time_ns
        LAST_RESULTS = res
    else:
        res = run_bass_kernel_spmd(nc, in_maps, list(range(NC)))
    return res.results[0]["y"].reshape(N_GRAPHS).astype(np.float32)
